# revision 23
# baseline (speedup 1.0000x reference)
"""Multi-head self-attention (B=2, S=2048, E=1024, H=16) on 8 Trainium2 cores.

Sharding: 2D (batch x head-group). Core c handles batch b = c // 4 and head
group g = c % 4 (4 heads, 256 embed columns). Each core computes its QKV
projection slices, fused attention for its 4 heads, and a partial output
projection (attn_g @ Wo[g_slice]); the host sums the 4 partials per batch
(the head-concat contraction) and stacks the 2 batches.

Device layout choices (all matmul contractions land on the partition axis,
so no on-device transposes are needed anywhere):
  - host supplies x^T per batch ([E, S], bf16) for q/k/v
  - Q/K projections produce Q^T/K^T  [d', S] (head-dim on partitions)
  - V projection produces V [S, d'] (seq on partitions), stored interleaved
    with a ones column per head ([V_h | 1] * 4) so that P @ [V_h | 1] yields
    both the attention numerator and the softmax denominator in one pass
  - logits^T tiles [j, i] feed exp (ScalarE, no max-subtraction: |logits|<~8)
    giving P^T tiles which are exactly the rhs layout P@V needs
  - 1/8 scaling and biases are folded in on the host / into copy-backs;
    bv is folded via P @ [V + 1 bv^T] = P@V + bv (softmax rows sum to 1)
"""

import numpy as np
import ml_dtypes

BF16 = ml_dtypes.bfloat16

P = 128
S = 2048
E = 1024
GE = 256          # embed columns per core (4 heads x 64)
KC = 8            # contraction chunks of 128 over E
JC = 16           # key chunks of 128 over S
IT = 4            # query tiles of 512 over S
NCORES = 8

_NC = None        # cached compiled program


def _build_program():
    import concourse.tile as tile
    from concourse import bacc, mybir

    F32 = mybir.dt.float32
    BF = mybir.dt.bfloat16
    Exp = mybir.ActivationFunctionType.Exp
    mult = mybir.AluOpType.mult
    add = mybir.AluOpType.add

    nc = bacc.Bacc(
        "TRN2",
        target_bir_lowering=False,
        debug=False,
        enable_asserts=False,
        num_devices=NCORES,
    )

    d_xq = nc.dram_tensor("xqT", [E, S], BF, kind="ExternalInput")
    d_xk = nc.dram_tensor("xkT", [E, S], BF, kind="ExternalInput")
    d_xv = nc.dram_tensor("xvT", [E, S], BF, kind="ExternalInput")
    d_wq = nc.dram_tensor("wq", [P, KC, GE], BF, kind="ExternalInput")
    d_wk = nc.dram_tensor("wk", [P, KC, GE], BF, kind="ExternalInput")
    d_wv = nc.dram_tensor("wv", [P, KC, GE], BF, kind="ExternalInput")
    d_wo = nc.dram_tensor("wo", [P, 2, E], BF, kind="ExternalInput")
    d_bq = nc.dram_tensor("bqs", [P, 2], F32, kind="ExternalInput")
    d_bk = nc.dram_tensor("bks", [P, 2], F32, kind="ExternalInput")
    d_bv = nc.dram_tensor("bvb", [P, GE], F32, kind="ExternalInput")
    d_bo = nc.dram_tensor("bob", [P, E], F32, kind="ExternalInput")
    d_y = nc.dram_tensor("y", [S, E], F32, kind="ExternalOutput")

    with tile.TileContext(nc) as tc:
        with (
            tc.tile_pool(name="w", bufs=1) as wpool,
            tc.tile_pool(name="x", bufs=1) as xpool,
            tc.tile_pool(name="persist", bufs=1) as pers,
            tc.tile_pool(name="pt", bufs=16) as ptp,
            tc.tile_pool(name="sm", bufs=2) as sm,
            tc.tile_pool(name="y", bufs=2) as yp,
            tc.tile_pool(name="psA", bufs=2, space="PSUM") as psA,
            tc.tile_pool(name="psB", bufs=4, space="PSUM") as psB,
            tc.tile_pool(name="dr", bufs=2, space="DRAM") as drp,
        ):
            # ---- weights / biases resident in SBUF ----
            wq_t = wpool.tile([P, KC, GE], BF, tag="wq")
            wk_t = wpool.tile([P, KC, GE], BF, tag="wk")
            wv_t = wpool.tile([P, KC, GE], BF, tag="wv")
            wo_t = wpool.tile([P, 2, E], BF, tag="wo")
            bq_t = wpool.tile([P, 2], F32, tag="bq")
            bk_t = wpool.tile([P, 2], F32, tag="bk")
            bv_t = wpool.tile([P, GE], F32, tag="bv")
            bo_t = wpool.tile([P, E], F32, tag="bo")

            for t, d in (
                (wq_t, d_wq), (wk_t, d_wk), (wv_t, d_wv), (wo_t, d_wo),
                (bq_t, d_bq), (bk_t, d_bk), (bv_t, d_bv), (bo_t, d_bo),
            ):
                nc.sync.dma_start(t[:], d[:])

            # ---- persistent activations ----
            QT = pers.tile([P, 2, S], BF, tag="QT")   # [d'(2x128), S]
            KT = pers.tile([P, 2, S], BF, tag="KT")
            V1 = pers.tile([P, JC, 260], BF, tag="V1")  # [S(16x128), (V_h|1)*4]
            OT = pers.tile([P, 2, S], BF, tag="OT")

            # ones columns (col 64 of each 65-wide head block)
            nc.vector.memset(V1[:, :, 64::65], 1.0)

            def load_chunks(xd):
                chs = []
                for kc in range(KC):
                    t = xpool.tile([P, S], BF, tag="xchunk", bufs=12)
                    nc.sync.dma_start(t[:], xd[kc * P:(kc + 1) * P, :])
                    chs.append(t)
                return chs

            xk = load_chunks(d_xk)
            xq = load_chunks(d_xq)
            xv = load_chunks(d_xv)

            def qk_group(w_t, b_t, dst, xch, c, tt, th):
                # one [128, 512] span of a Q/K projection (8 accumulating MMs
                # + biased copy-back); psum from the 1-bank rotating pool so
                # the logits double-buffer is never starved.
                ps = psB.tile([P, 512], F32, tag="acc")
                s0 = (tt * 2 + th) * 512
                for kc in range(KC):
                    nc.tensor.matmul(
                        ps[:],
                        lhsT=w_t[:, kc, c * P:(c + 1) * P],
                        rhs=xch[kc][:, s0:s0 + 512],
                        start=(kc == 0), stop=(kc == KC - 1),
                    )
                nc.vector.tensor_scalar_add(
                    dst[:, c, s0:s0 + 512], ps[:], b_t[:, c:c + 1])

            def qk_fill_group(w_t, b_t, dst, xd, c, tt, th):
                # same as qk_group, but re-loads its x slices from DRAM in one
                # strided DMA so the full x chunks don't stay SBUF-resident
                s0 = (tt * 2 + th) * 512
                xs = xpool.tile([P, KC, 512], BF, tag="xs", bufs=3)
                nc.sync.dma_start(
                    xs[:],
                    xd[:, s0:s0 + 512].rearrange("(kc p) s -> p kc s", p=P))
                ps = psB.tile([P, 512], F32, tag="acc")
                for kc in range(KC):
                    nc.tensor.matmul(
                        ps[:],
                        lhsT=w_t[:, kc, c * P:(c + 1) * P],
                        rhs=xs[:, kc, :],
                        start=(kc == 0), stop=(kc == KC - 1),
                    )
                nc.vector.tensor_scalar_add(
                    dst[:, c, s0:s0 + 512], ps[:], b_t[:, c:c + 1])

            def v_group(sg):
                # V projection for two s-chunks -> V1 (interleaved V|1 cols)
                ps = psB.tile([P, 512], F32, tag="acc")
                for i2 in range(2):
                    sc = sg * 2 + i2
                    for kc in range(KC):
                        nc.tensor.matmul(
                            ps[:, i2 * GE:(i2 + 1) * GE],
                            lhsT=xv[kc][:, sc * P:(sc + 1) * P],
                            rhs=wv_t[:, kc, :],
                            start=(kc == 0), stop=(kc == KC - 1),
                        )
                for i2 in range(2):
                    sc = sg * 2 + i2
                    for h in range(4):
                        nc.vector.tensor_tensor(
                            V1[:, sc, 65 * h:65 * h + 64],
                            ps[:, i2 * GE + 64 * h:i2 * GE + 64 * (h + 1)],
                            bv_t[:, 64 * h:64 * (h + 1)],
                            add,
                        )

            def out_group(sc, nt, ysb):
                ps = psB.tile([P, 512], F32, tag="acc")
                for cc in range(2):
                    nc.tensor.matmul(
                        ps[:],
                        lhsT=OT[:, cc, sc * P:(sc + 1) * P],
                        rhs=wo_t[:, cc, nt * 512:(nt + 1) * 512],
                        start=(cc == 0), stop=(cc == 1),
                    )
                nc.vector.tensor_tensor(
                    ysb[:, nt * 512:(nt + 1) * 512], ps[:],
                    bo_t[:, nt * 512:(nt + 1) * 512], add)

            # ---- prologue: K (both halves), Q for c=0, first V group;
            # the remaining V groups ride as unit-(0,0) fillers ----
            for tt in range(2):
                for th in range(2):
                    qk_group(wk_t, bk_t, KT, xk, 0, tt, th)
            for tt in range(2):
                for th in range(2):
                    qk_group(wq_t, bq_t, QT, xq, 0, tt, th)
            v_group(0)

            # ---- attention, ACT(exp)-bound; remaining projection and the
            # output-projection work is sprinkled between jc iterations so it
            # fills the PE's idle capacity without starving the exp stream ----
            def attn_unit(c, t, fillers):
                tsl = slice(t * 512, (t + 1) * 512)
                pO0 = psB.tile([65, 512], F32, tag="acc")
                pO1 = psB.tile([65, 512], F32, tag="acc")
                for jc in range(JC):
                    jsl = slice(jc * P, (jc + 1) * P)
                    pL = psA.tile([P, 1024], F32, tag="big")
                    nc.tensor.matmul(
                        pL[:, 0:512],
                        lhsT=KT[0:64, c, jsl], rhs=QT[0:64, c, tsl],
                        start=True, stop=True,
                    )
                    nc.tensor.matmul(
                        pL[:, 512:1024],
                        lhsT=KT[64:128, c, jsl], rhs=QT[64:128, c, tsl],
                        start=True, stop=True,
                    )
                    pt = ptp.tile([P, 1024], BF, tag="pt")
                    nc.scalar.activation(pt[:], pL[:], Exp)
                    nc.tensor.matmul(
                        pO0[:], lhsT=V1[:, jc, 130 * c:130 * c + 65],
                        rhs=pt[:, 0:512],
                        start=(jc == 0), stop=(jc == JC - 1),
                    )
                    nc.tensor.matmul(
                        pO1[:], lhsT=V1[:, jc, 130 * c + 65:130 * c + 130],
                        rhs=pt[:, 512:1024],
                        start=(jc == 0), stop=(jc == JC - 1),
                    )
                    if jc % 2 == 1 and fillers:
                        fillers.pop(0)()
                # normalize: OT_h = pO[0:64] / pO[64]  (row 64 = sum of P).
                # First copy the accumulator out of PSUM so its bank frees
                # immediately; then, with only standard instructions: spread
                # the 512 sums over 64 partitions (exact reciprocal becomes
                # 8 elem/lane) and broadcast 1/S back via a DRAM bounce
                # (DRAM APs may have partition-step 0). Chain DMAs ride the
                # mostly-idle GpSimd queue.
                for hp, pO in ((0, pO0), (1, pO1)):
                    osb = sm.tile([65, 512], F32, tag="osb", bufs=3)
                    nc.vector.tensor_copy(osb[:], pO[:])
                    drT = drp.tile([512], F32, tag="drT")
                    nc.gpsimd.dma_start(drT[None, :], osb[64:65, :])
                    r64 = sm.tile([64, 8], F32, tag="r64", bufs=3)
                    nc.gpsimd.dma_start(
                        r64[:], drT[:].rearrange("(p f) -> p f", p=64))
                    r64r = sm.tile([64, 8], F32, tag="r64r", bufs=3)
                    nc.vector.reciprocal(r64r[:], r64[:])
                    drS = drp.tile([512], F32, tag="drS")
                    nc.gpsimd.dma_start(drS[:].rearrange("(p f) -> p f", p=64),
                                        r64r[:])
                    rbs = sm.tile([64, 512], F32, tag="rbs", bufs=3)
                    nc.gpsimd.dma_start(rbs[:],
                                        drS[None, :].to_broadcast((64, 512)))
                    ott = sm.tile([64, 512], BF, tag="ott", bufs=3)
                    nc.vector.tensor_tensor(ott[:], osb[0:64, :],
                                            rbs[:], mult)
                    nc.gpsimd.dma_start(OT[64 * hp:64 * (hp + 1), c, tsl],
                                        ott[:])
                while fillers:
                    fillers.pop(0)()

            def mkqk(w_t, b_t, dst, xd, c, tt, th):
                return lambda: qk_fill_group(w_t, b_t, dst, xd, c, tt, th)

            ysbs = {}

            def mkout(sc, nt):
                def f():
                    if sc not in ysbs:
                        ysbs[sc] = yp.tile([P, E], F32, tag="ysb",
                                           name=f"ysb_{sc}")
                    out_group(sc, nt, ysbs[sc])
                    if nt == 1:
                        nc.sync.dma_start(
                            d_y[sc * P:(sc + 1) * P, :], ysbs.pop(sc)[:])
                return f

            # c=0 units carry the remaining projections; c=1 units carry the
            # output projection of the PREVIOUS finished i-tile (Tile orders
            # dataflow by emission, so a filler may only read regions whose
            # writes were already emitted).
            def outfill(t):
                return [mkout(sc, nt)
                        for sc in range(4 * t, 4 * t + 4) for nt in range(2)]

            attn_unit(0, 0, [(lambda g=g: v_group(g)) for g in range(1, 8)])
            attn_unit(0, 1, [mkqk(wk_t, bk_t, KT, d_xk, 1, 0, 0),
                             mkqk(wk_t, bk_t, KT, d_xk, 1, 0, 1)])
            attn_unit(0, 2, [mkqk(wk_t, bk_t, KT, d_xk, 1, 1, 0),
                             mkqk(wk_t, bk_t, KT, d_xk, 1, 1, 1)])
            attn_unit(0, 3, [mkqk(wq_t, bq_t, QT, d_xq, 1, 0, 0),
                             mkqk(wq_t, bq_t, QT, d_xq, 1, 0, 1)])
            attn_unit(1, 0, [mkqk(wq_t, bq_t, QT, d_xq, 1, 1, 0),
                             mkqk(wq_t, bq_t, QT, d_xq, 1, 1, 1)])
            attn_unit(1, 1, outfill(0))
            attn_unit(1, 2, outfill(1))
            attn_unit(1, 3, outfill(2))
            for f in outfill(3):
                f()

    nc.compile()
    return nc


def _get_program():
    global _NC
    if _NC is None:
        _NC = _build_program()
    return _NC


def kernel(q, k, v, Wq, bq, Wk, bk, Wv, bv, Wo, bo):
    from concourse.bass_utils import run_bass_kernel_spmd

    q = np.asarray(q, np.float32)
    k = np.asarray(k, np.float32)
    v = np.asarray(v, np.float32)
    Wq = np.asarray(Wq, np.float32)
    Wk = np.asarray(Wk, np.float32)
    Wv = np.asarray(Wv, np.float32)
    Wo = np.asarray(Wo, np.float32)
    bq = np.asarray(bq, np.float32)
    bk = np.asarray(bk, np.float32)
    bv = np.asarray(bv, np.float32)
    bo = np.asarray(bo, np.float32)

    nc = _get_program()

    xT = {name: [np.ascontiguousarray(x[b].T).astype(BF16) for b in range(2)]
          for name, x in (("xqT", q), ("xkT", k), ("xvT", v))}

    def wprep(W, scale=1.0):
        # [E, GE] slice -> [P, KC, GE] partition-major
        return [
            np.ascontiguousarray(
                (W[:, g * GE:(g + 1) * GE] * scale)
                .reshape(KC, P, GE).transpose(1, 0, 2)
            ).astype(BF16)
            for g in range(4)
        ]

    wq_g = wprep(Wq, 0.125)
    wk_g = wprep(Wk)
    wv_g = wprep(Wv)
    wo_g = [
        np.ascontiguousarray(
            Wo[g * GE:(g + 1) * GE, :].reshape(2, P, E).transpose(1, 0, 2)
        ).astype(BF16)
        for g in range(4)
    ]
    bq_g = [np.ascontiguousarray((bq[g * GE:(g + 1) * GE] * 0.125)
                                 .reshape(2, P).T).astype(np.float32)
            for g in range(4)]
    bk_g = [np.ascontiguousarray(bk[g * GE:(g + 1) * GE].reshape(2, P).T)
            .astype(np.float32) for g in range(4)]
    bv_g = [np.ascontiguousarray(np.broadcast_to(
        bv[g * GE:(g + 1) * GE].astype(np.float32), (P, GE))) for g in range(4)]
    bo_full = np.ascontiguousarray(
        np.broadcast_to(bo.astype(np.float32), (P, E)))
    bo_zero = np.zeros((P, E), np.float32)

    in_maps = []
    for c in range(NCORES):
        b, g = divmod(c, 4)
        in_maps.append({
            "xqT": xT["xqT"][b],
            "xkT": xT["xkT"][b],
            "xvT": xT["xvT"][b],
            "wq": wq_g[g], "wk": wk_g[g], "wv": wv_g[g], "wo": wo_g[g],
            "bqs": bq_g[g], "bks": bk_g[g], "bvb": bv_g[g],
            "bob": bo_full if g == 0 else bo_zero,
        })

    res = run_bass_kernel_spmd(nc, in_maps, list(range(NCORES)),
                               **_RUN_KWARGS)
    globals()["LAST_RESULTS"] = res

    parts = [res.results[c]["y"] for c in range(NCORES)]
    out = np.stack([
        parts[0] + parts[1] + parts[2] + parts[3],
        parts[4] + parts[5] + parts[6] + parts[7],
    ]).astype(np.float32)
    return out


# test-harness hooks (kernel.py itself never enables tracing)
_RUN_KWARGS = {}
LAST_RESULTS = None
